# revision 15
# baseline (speedup 1.0000x reference)
"""Tacotron2-style decoder on 8 Trainium2 NeuronCores.

Strategy (chosen over the data-parallel hint): tensor-parallel over the 4096
LSTM gate dims (512 gates/core) with weights resident in SBUF, attention
row-parallel (4 batch rows/core), two AllGathers per step (h_a; ctx).
Data-parallel would stream all 73MB of weights from HBM every step on every
core (they don't fit in SBUF) — memory-catastrophic for a 250-step recurrence.

kernel(**inputs) -> (mel_outputs, stop_tokens, attn_scores), matching
reference.reference().
"""
import numpy as np

import concourse.bass as bass
import concourse.mybir as mybir
import concourse.tile as tile
from concourse import bacc
from concourse import bass_utils

F32 = mybir.dt.float32
F32R = mybir.dt.float32r
AF = mybir.ActivationFunctionType

B, T_ENC, ENC_D = 32, 256, 512
T_DEC, MEL, R = 500, 80, 2
PRE = 256
ATTN_D, ATTN_RNN, DEC_RNN = 128, 1024, 1024
KSIZE = 31
PAD = (KSIZE - 1) // 2
NC = 8
RB = B // NC            # 4 rows per core
GS = 4 * ATTN_RNN // NC  # 512 gates per core
HS = ATTN_RNN // NC      # 128 h-dims per core
N_STEPS = T_DEC // R     # 250


def _r(x):  # operands are natively f32r now
    return x


def build_nc(n_steps):
    nc = bacc.Bacc("TRN2", target_bir_lowering=False, debug=False,
                   num_devices=NC)
    RG = [list(range(NC))]

    def inp(name, shape, dtype=F32):
        return nc.dram_tensor(name, list(shape), dtype, kind="ExternalInput")

    xT = inp("xT", (n_steps, 2, 128, B), F32R)
    WihaT = inp("WihaT", (128, 6, GS), F32R)
    WhhaT = inp("WhhaT", (128, 8, GS), F32R)
    WihdT = inp("WihdT", (128, 12, GS), F32R)
    WhhdT = inp("WhhdT", (128, 8, GS), F32R)
    ba_i = inp("ba", (1, GS), F32R)
    bd_i = inp("bd", (1, GS), F32R)
    WqT_i = inp("WqT", (128, 8, ATTN_D), F32R)
    WmsT_i = inp("WmsT", (128, 12, 162), F32R)
    bms_i = inp("bms", (1, 162), F32R)
    U31_i = inp("U31", (KSIZE, 128), F32R)
    wlcb_i = inp("wlcb", (128, 1))
    v_i = inp("vcol", (128, 2), F32R)
    enc_i = inp("enc_sb", (128, 8, ENC_D), F32R)
    pmT_i = inp("pmT", (128, RB, T_ENC))
    mask_i = inp("maskadd", (128, T_ENC))
    SEL_i = inp("SEL", (B, RB), F32R)
    ones_i = inp("ones", (1, B), F32R)
    ident_i = inp("ident", (128, 128))
    zz_i = inp("zz", (128, B), F32R)

    mel_o = nc.dram_tensor("mel_o", [n_steps, B, 161], F32,
                           kind="ExternalOutput")
    attn_o = nc.dram_tensor("attn_o", [n_steps, RB, T_ENC], F32,
                            kind="ExternalOutput")

    with tile.TileContext(nc) as tc:
        from contextlib import ExitStack
        stack = ExitStack()
        const_pool = stack.enter_context(tc.tile_pool(name="const", bufs=1))

        # ---------------- persistent SBUF ----------------
        def persist(name, shape, dtype=F32):
            return const_pool.tile(list(shape), dtype, name=name, tag=name)

        wiha = persist("wiha", (128, 6, GS), F32R)
        whha = persist("whha", (128, 8, GS), F32R)
        wihd = persist("wihd", (128, 12, GS), F32R)
        whhd = persist("whhd", (128, 8, GS), F32R)
        ba = persist("ba_s", (1, GS), F32R)
        bd = persist("bd_s", (1, GS), F32R)
        wqt = persist("wqt", (128, 8, ATTN_D), F32R)
        wmst = persist("wmst", (128, 12, 162), F32R)
        bms = persist("bms_s", (1, 162), F32R)
        u31 = persist("u31", (KSIZE, 128), F32R)
        wlcb = persist("wlcb_s", (128, 1))
        vcol = persist("vcol_s", (128, 2), F32R)
        enc = persist("enc_s", (128, 8, ENC_D), F32R)
        pmt = persist("pmt", (128, RB, T_ENC))
        mask = persist("mask_s", (128, T_ENC))
        sel = persist("sel_s", (B, RB), F32R)
        ones = persist("ones_s", (1, B), F32R)
        ident = persist("ident_s", (128, 128))
        zz = persist("zz_s", (128, B), F32R)

        haT = [persist(f"haT{i}", (128, 8, B), F32R) for i in range(2)]
        hdT = [persist(f"hdT{i}", (128, 8, B), F32R) for i in range(2)]
        ctxT = [persist(f"ctxT{i}", (128, RB, B), F32R) for i in range(2)]
        c_a = [persist(f"c_a{i}", (B, HS)) for i in range(2)]
        c_d = [persist(f"c_d{i}", (B, HS)) for i in range(2)]
        blk = [persist(f"blk{i}", (128, B), F32R) for i in range(2)]
        cum = persist("cum", (RB, T_ENC + 2 * PAD), F32R)

        # prologue loads
        for dst, src in [(wiha, WihaT), (whha, WhhaT), (wihd, WihdT),
                         (whhd, WhhdT), (ba, ba_i), (bd, bd_i),
                         (wqt, WqT_i), (wmst, WmsT_i), (bms, bms_i),
                         (u31, U31_i), (wlcb, wlcb_i), (vcol, v_i),
                         (enc, enc_i), (pmt, pmT_i), (mask, mask_i),
                         (sel, SEL_i), (ones, ones_i), (ident, ident_i),
                         (zz, zz_i)]:
            nc.sync.dma_start(dst[:], src[:])
        for t2 in haT + hdT + ctxT + c_a + c_d + blk + [cum]:
            nc.vector.memset(t2[:].bitcast(F32), 0.0)

        # ---------------- pools ----------------
        sb = stack.enter_context(tc.tile_pool(name="sb", bufs=2))
        sb3 = stack.enter_context(tc.tile_pool(name="sb3", bufs=3))
        ps_g = stack.enter_context(tc.tile_pool(name="ps_g", bufs=1,
                                                space="PSUM"))
        ps_att = stack.enter_context(tc.tile_pool(name="ps_att", bufs=1,
                                                  space="PSUM"))
        ps_sm = stack.enter_context(tc.tile_pool(name="ps_sm", bufs=1,
                                                 space="PSUM"))
        ps_eps = stack.enter_context(tc.tile_pool(name="ps_eps", bufs=2,
                                                  space="PSUM"))
        dram = stack.enter_context(tc.tile_pool(name="dram", bufs=3,
                                                space="DRAM"))

        ID32 = ident[0:32, 0:32]
        ID4 = ident[0:RB, 0:RB]

        def lstm_nl(gpsum, c_old, c_new, tag):
            """gates psum [B, 4*HS] -> h [B,HS] sbuf, c_new written."""
            i_s = sb.tile([B, HS], F32, name=f"i_{tag}", tag=f"i_{tag}")
            f_s = sb.tile([B, HS], F32, name=f"f_{tag}", tag=f"f_{tag}")
            g_t = sb.tile([B, HS], F32, name=f"g_{tag}", tag=f"g_{tag}")
            o_s = sb.tile([B, HS], F32, name=f"o_{tag}", tag=f"o_{tag}")
            tc_t = sb.tile([B, HS], F32, name=f"tc_{tag}", tag=f"tc_{tag}")
            h_sb = sb.tile([B, HS], F32, name=f"h_{tag}", tag=f"h_{tag}")
            nc.scalar.activation(i_s[:], gpsum[:, 0:HS], AF.Sigmoid)
            nc.scalar.activation(f_s[:], gpsum[:, HS:2 * HS], AF.Sigmoid)
            nc.scalar.activation(g_t[:], gpsum[:, 2 * HS:3 * HS], AF.Tanh)
            nc.scalar.activation(o_s[:], gpsum[:, 3 * HS:4 * HS], AF.Sigmoid)
            nc.vector.tensor_mul(f_s[:], f_s[:], c_old[:])
            nc.vector.tensor_mul(i_s[:], i_s[:], g_t[:])
            nc.vector.tensor_add(c_new[:], f_s[:], i_s[:])
            nc.scalar.activation(tc_t[:], c_new[:], AF.Tanh)
            nc.vector.tensor_mul(h_sb[:], o_s[:], tc_t[:])
            return h_sb

        agin1_next = dram.tile([2, 128, B], F32R, name="agin1", tag="agin1")
        nc.sync.dma_start(agin1_next[1], zz[:])   # h_d(-1) = 0

        for t in range(n_steps + 1):
            p, q = t % 2, (t + 1) % 2   # q = previous parity
            last = t == n_steps

            # ---- ploc precompute (needs cum(t), ready since t-1) ----
            if not last:
                cwin = sb.tile([KSIZE, RB, T_ENC], F32R, name="cwin",
                               tag="cwin")
                for j in range(RB):
                    base = cum[j:j + 1, 0:T_ENC]
                    src = bass.AP(base.tensor, base.offset,
                                  [list(base.ap[0]), [1, KSIZE], [1, T_ENC]])
                    nc.sync.dma_start(cwin[:, j, :], src)
                s_sb = sb.tile([128, RB, T_ENC], F32, name="s_sb", tag="s_sb")
                for h in range(2):
                    ploc = ps_att.tile([128, 2, T_ENC], F32, name="ploc",
                                       tag="ploc")
                    for jj in range(2):
                        j = 2 * h + jj
                        nc.tensor.matmul(ploc[:, jj, :], _r(u31[:]),
                                         _r(cwin[:, j, :]), start=True,
                                         stop=True)
                    for jj in range(2):
                        j = 2 * h + jj
                        nc.vector.tensor_add(s_sb[:, j, :], ploc[:, jj, :],
                                             pmt[:, j, :])

                # ---- gates_a ----
                g_a = ps_g.tile([B, GS], F32, name="g_a", tag="g_a")
                xts = sb3.tile([128, 2, B], F32R, name="xts", tag="xts")
                nc.sync.dma_start(xts[:], xT[t].transpose([1, 0, 2]))
                mm = []
                for k in range(2):
                    mm.append((xts[:, k, :], wiha[:, k, :]))
                for k in range(4):
                    mm.append((ctxT[q][:, k, :], wiha[:, 2 + k, :]))
                for k in range(8):
                    mm.append((haT[q][:, k, :], whha[:, k, :]))
                mm.append((ones[:], ba[:]))
                for i, (l, r_) in enumerate(mm):
                    nc.tensor.matmul(g_a[:], _r(l), _r(r_), start=(i == 0),
                                     stop=(i == len(mm) - 1))
                h_a = lstm_nl(g_a, c_a[q], c_a[p], "a")

                # transpose h_a -> [128, B], ship to AG1
                haTp = ps_sm.tile([128, B], F32, name="haTp", tag="sm")
                nc.tensor.transpose(haTp[:], h_a[:], ID32)
                haT_sb = sb.tile([128, B], F32R, name="haT_sb", tag="haT_sb")
                nc.vector.tensor_copy(haT_sb[:], haTp[:])
                agin1 = agin1_next
                nc.sync.dma_start(agin1[0], haT_sb[:])

                agout1 = dram.tile([NC, 2, 128, B], F32R, name="agout1",
                                   tag="agout1")
                nc.gpsimd.collective_compute(
                    "AllGather", mybir.AluOpType.bypass, replica_groups=RG,
                    ins=[agin1[:].opt()], outs=[agout1[:].opt()])
                nc.sync.dma_start(haT[p][:],
                                  agout1[:, 0, :, :].transpose([1, 0, 2]))
                nc.sync.dma_start(hdT[p][:],
                                  agout1[:, 1, :, :].transpose([1, 0, 2]))
            else:
                agin1 = agin1_next
                nc.sync.dma_start(agin1[0], zz[:])
                agout1 = dram.tile([NC, 2, 128, B], F32R, name="agout1",
                                   tag="agout1")
                nc.gpsimd.collective_compute(
                    "AllGather", mybir.AluOpType.bypass, replica_groups=RG,
                    ins=[agin1[:].opt()], outs=[agout1[:].opt()])
                nc.sync.dma_start(hdT[p][:],
                                  agout1[:, 1, :, :].transpose([1, 0, 2]))

            # ---- deferred mel/stop for step t-1 ----
            if t > 0:
                melp = ps_sm.tile([B, 162], F32, name="melp", tag="melp", bufs=1)
                mmm = [(hdT[p][:, k, :], wmst[:, k, :]) for k in range(8)]
                mmm += [(ctxT[q][:, k, :], wmst[:, 8 + k, :])
                        for k in range(4)]
                mmm.append((ones[:], bms[:]))
                for i, (l, r_) in enumerate(mmm):
                    nc.tensor.matmul(melp[:], _r(l), _r(r_), start=(i == 0),
                                     stop=(i == len(mmm) - 1))
                mel_sb = sb.tile([B, 162], F32, name="mel_sb", tag="mel_sb")
                nc.vector.tensor_copy(mel_sb[:, 0:160], melp[:, 0:160])
                nc.scalar.activation(mel_sb[:, 160:161], melp[:, 160:161],
                                     AF.Sigmoid)
                nc.sync.dma_start(mel_o[t - 1], mel_sb[:, 0:161])
            if last:
                break

            # ---- attention (own 4 rows) ----
            pqp = ps_sm.tile([B, ATTN_D], F32, name="pqp", tag="sm")
            for j in range(8):
                nc.tensor.matmul(pqp[:], _r(haT[p][:, j, :]),
                                 _r(wqt[:, j, :]), start=(j == 0),
                                 stop=(j == 7))
            pq_sb = sb.tile([B, ATTN_D], F32R, name="pq_sb", tag="pq_sb")
            nc.vector.tensor_copy(pq_sb[:], pqp[:])
            pqo = ps_sm.tile([RB, ATTN_D], F32, name="pqo", tag="sm")
            nc.tensor.matmul(pqo[:], _r(sel[:]), _r(pq_sb[:]), start=True,
                             stop=True)
            pqo_sb = sb.tile([RB, ATTN_D], F32, name="pqo_sb", tag="pqo_sb")
            nc.vector.tensor_copy(pqo_sb[:], pqo[:])
            pqT = ps_sm.tile([128, RB], F32, name="pqT", tag="sm")
            nc.tensor.transpose(pqT[:], pqo_sb[:], ID4)
            bias = sb.tile([128, RB], F32, name="bias", tag="bias")
            nc.vector.tensor_scalar_add(bias[:], pqT[:], wlcb[:])

            tanh_s = sb.tile([128, RB, T_ENC], F32R, name="tanh_s",
                             tag="tanh_s")
            for j in range(RB):
                nc.scalar.activation(tanh_s[:, j, :], s_sb[:, j, :], AF.Tanh,
                                     bias=bias[:, j:j + 1])
            e_sbB = sb.tile([128, T_ENC], F32, name="e_sbB", tag="e_sbB")
            for j in range(RB):
                e_ps = ps_eps.tile([2, T_ENC], F32, name="e_ps", tag="eps")
                nc.tensor.matmul(e_ps[:], _r(vcol[:]),
                                 _r(tanh_s[:, j, :]), start=True, stop=True)
                nc.vector.tensor_add(e_sbB[32 * j:32 * j + 1, :],
                                     e_ps[0:1, :],
                                     mask[32 * j:32 * j + 1, :])
            e_sb = sb.tile([RB, T_ENC], F32, name="e_sb", tag="e_sb")
            _eap = e_sbB[:]
            _gsrc = bass.AP(_eap.tensor, _eap.offset,
                            [[_eap.ap[0][0] * 32, RB], [1, T_ENC]])
            nc.sync.dma_start(e_sb[:], _gsrc)
            nmx = sb.tile([RB, 1], F32, name="nmx", tag="nmx")
            nc.vector.tensor_reduce(nmx[:], e_sb[:], mybir.AxisListType.X,
                                    mybir.AluOpType.max, negate=True)
            exp_sb = sb.tile([RB, T_ENC], F32, name="exp_sb", tag="exp_sb")
            esum = sb.tile([RB, 1], F32, name="esum", tag="esum")
            nc.scalar.activation(exp_sb[:], e_sb[:], AF.Exp, bias=nmx[:],
                                 accum_out=esum[:])
            rcp = sb.tile([RB, 1], F32, name="rcp", tag="rcp")
            nc.vector.reciprocal(rcp[:], esum[:])
            align = sb.tile([RB, T_ENC], F32, name="align", tag="align")
            nc.vector.tensor_scalar_mul(align[:], exp_sb[:], rcp[:])
            nc.sync.dma_start(attn_o[t], align[:])
            nc.vector.tensor_add(cum[:, PAD:PAD + T_ENC],
                                 cum[:, PAD:PAD + T_ENC], align[:])

            alT = ps_sm.tile([128, 2 * RB], F32, name="alT", tag="sm")
            for th in range(2):
                nc.tensor.transpose(alT[:, th * RB:(th + 1) * RB],
                                    align[:, th * 128:(th + 1) * 128], ID4)
            for j in range(8):
                b_i, th = j // 2, j % 2
                nc.vector.tensor_copy(blk[p][:, j * RB + b_i:j * RB + b_i + 1],
                                      alT[:, th * RB + b_i:th * RB + b_i + 1])
            ctxp = ps_att.tile([RB, ENC_D], F32, name="ctxp", tag="ctxp")
            for j in range(8):
                nc.tensor.matmul(ctxp[:], _r(blk[p][:, j * RB:(j + 1) * RB]),
                                 _r(enc[:, j, :]), start=(j == 0),
                                 stop=(j == 7))

            ctx_sb = sb.tile([RB, ENC_D], F32R, name="ctx_sb", tag="ctx_sb")
            nc.vector.tensor_copy(ctx_sb[:], ctxp[:])
            agin2 = dram.tile([RB, ENC_D], F32R, name="agin2", tag="agin2")
            nc.sync.dma_start(agin2[:], ctx_sb[:])
            agout2 = dram.tile([NC, RB, ENC_D], F32R, name="agout2",
                               tag="agout2")
            nc.gpsimd.collective_compute(
                "AllGather", mybir.AluOpType.bypass, replica_groups=RG,
                ins=[agin2[:].opt()], outs=[agout2[:].opt()])
            # ctxT[p][pp, dj, r*RB+b] = agout2[r, b, dj*128+pp]
            for k in range(4):
                nc.sync.dma_start(
                    ctxT[p][:, k, :],
                    agout2[:, :, 128 * k:128 * (k + 1)]
                    .rearrange("r b pp -> pp (r b)"))

            # ---- gates_d ----
            g_d = ps_g.tile([B, GS], F32, name="g_d", tag="g_d")
            mm = [(haT[p][:, k, :], wihd[:, k, :]) for k in range(8)]
            mm += [(ctxT[p][:, k, :], wihd[:, 8 + k, :]) for k in range(4)]
            mm += [(hdT[p][:, k, :], whhd[:, k, :]) for k in range(8)]
            mm.append((ones[:], bd[:]))
            for i, (l, r_) in enumerate(mm):
                nc.tensor.matmul(g_d[:], _r(l), _r(r_), start=(i == 0),
                                 stop=(i == len(mm) - 1))
            h_d = lstm_nl(g_d, c_d[q], c_d[p], "d")
            hdTp = ps_sm.tile([128, B], F32, name="hdTp", tag="sm")
            nc.tensor.transpose(hdTp[:], h_d[:], ID32)
            hdT_sb = sb.tile([128, B], F32R, name="hdT_sb", tag="hdT_sb")
            nc.vector.tensor_copy(hdT_sb[:], hdTp[:])
            agin1_next = dram.tile([2, 128, B], F32R, name="agin1",
                                   tag="agin1")
            nc.sync.dma_start(agin1_next[1], hdT_sb[:])

        stack.close()

    nc.compile()
    return nc


# ---------------------------------------------------------------------------
# host-side prep
# ---------------------------------------------------------------------------
def prepare_in_maps(inputs, n_steps):
    f32 = np.float32

    def A(x):
        return np.ascontiguousarray(np.asarray(x, dtype=f32))

    enc_full = A(inputs['encoder_outputs'])
    x_in = A(inputs['inputs'])
    mlen = np.asarray(inputs['memory_lengths'])

    frames = np.zeros((N_STEPS, B, MEL), f32)
    frames[1:] = x_in[:, R - 1:T_DEC - 1:R, :].transpose(1, 0, 2)
    x = np.maximum(frames @ A(inputs['W1']).T + A(inputs['b1']), 0)
    x = np.maximum(x @ A(inputs['W2']).T + A(inputs['b2']), 0)
    xT = np.ascontiguousarray(
        x.transpose(0, 2, 1).reshape(N_STEPS, 2, 128, B)[:n_steps])

    def shard_rows(W, c):
        return np.concatenate(
            [W[q * ATTN_RNN + c * HS:q * ATTN_RNN + (c + 1) * HS]
             for q in range(4)], 0)

    Wih_a, Whh_a = A(inputs['Wih_a']), A(inputs['Whh_a'])
    Wih_d, Whh_d = A(inputs['Wih_d']), A(inputs['Whh_d'])
    ba_full = A(inputs['bih_a']) + A(inputs['bhh_a'])
    bd_full = A(inputs['bih_d']) + A(inputs['bhh_d'])
    Wm, WL = A(inputs['Wm']), A(inputs['WL'])
    conv_w, conv_b = A(inputs['conv_w']), A(inputs['conv_b'])
    Wq, v = A(inputs['Wq']), A(inputs['v'])
    Wms = np.concatenate([A(inputs['Wmel']), A(inputs['Wstop']),
                          np.zeros((1, 1536), f32)], 0)
    bms = np.concatenate([A(inputs['bmel']), A(inputs['bstop']),
                          np.zeros(1, f32)])[None]

    U31 = np.ascontiguousarray((WL @ conv_w[:, 0, :]).T)       # (31, 128)
    wlcb = np.ascontiguousarray((WL @ conv_b)[:, None])        # (128, 1)
    WqT = np.ascontiguousarray(
        Wq.T.reshape(8, 128, ATTN_D).transpose(1, 0, 2))       # (128,8,128)
    WmsT = np.ascontiguousarray(
        Wms.T.reshape(12, 128, 162).transpose(1, 0, 2))        # (128,12,162)

    shared = dict(xT=xT, ba=None, bd=None, WqT=WqT, WmsT=WmsT, bms=bms,
                  U31=U31, wlcb=wlcb, vcol=np.ascontiguousarray(np.concatenate([v[0][:, None], np.zeros((128, 1), f32)], 1)),
                  ones=np.ones((1, B), f32), ident=np.eye(128, dtype=f32),
                  zz=np.zeros((128, B), f32))

    in_maps = []
    for c in range(NC):
        rows = slice(c * RB, (c + 1) * RB)
        enc_own = enc_full[rows]
        enc_sb = np.ascontiguousarray(
            enc_own.reshape(RB * T_ENC, ENC_D).reshape(8, 128, ENC_D)
                   .transpose(1, 0, 2))
        pm = enc_own @ Wm.T                                    # (4,256,128)
        pmT = np.ascontiguousarray(pm.transpose(2, 0, 1))      # (128,4,256)
        mrows = np.where(
            np.arange(T_ENC)[None, :] >= np.asarray(mlen[rows])[:, None],
            np.float32(-1e9), np.float32(0.0)).astype(f32)
        maskadd = np.zeros((128, T_ENC), f32)
        maskadd[::32][:RB] = mrows
        SEL = np.zeros((B, RB), f32)
        for j in range(RB):
            SEL[c * RB + j, j] = 1.0

        def wt(W, nch):
            s = shard_rows(W, c)
            return np.ascontiguousarray(
                s.T.reshape(nch, 128, GS).transpose(1, 0, 2))

        m = dict(shared)
        m['WihaT'] = wt(Wih_a, 6)
        m['WhhaT'] = wt(Whh_a, 8)
        m['WihdT'] = wt(Wih_d, 12)
        m['WhhdT'] = wt(Whh_d, 8)
        m['ba'] = np.ascontiguousarray(np.concatenate(
            [ba_full[qq * ATTN_RNN + c * HS:qq * ATTN_RNN + (c + 1) * HS]
             for qq in range(4)])[None])
        m['bd'] = np.ascontiguousarray(np.concatenate(
            [bd_full[qq * ATTN_RNN + c * HS:qq * ATTN_RNN + (c + 1) * HS]
             for qq in range(4)])[None])
        m['enc_sb'] = enc_sb
        m['pmT'] = pmT
        m['maskadd'] = maskadd
        m['SEL'] = SEL
        in_maps.append(m)
    return in_maps


_CACHE = {}


def _get_nc(n_steps):
    if n_steps not in _CACHE:
        _CACHE[n_steps] = build_nc(n_steps)
    return _CACHE[n_steps]


def run(inputs, n_steps=N_STEPS, trace=False):
    nc = _get_nc(n_steps)
    in_maps = prepare_in_maps(inputs, n_steps)
    res = bass_utils.run_bass_kernel_spmd(
        nc, in_maps, core_ids=list(range(NC)), trace=trace)
    r = res.results
    mel_ms = r[0]['mel_o']                       # (n, 32, 161)
    mel = np.ascontiguousarray(
        mel_ms[:, :, :160].transpose(1, 0, 2).reshape(B, n_steps * R, MEL))
    stop = np.repeat(mel_ms[:, :, 160].T, R, axis=1)
    attn = np.concatenate([r[c]['attn_o'] for c in range(NC)], axis=1)
    attn = np.ascontiguousarray(attn.transpose(1, 0, 2))
    return (mel, stop, attn), res


def kernel(**inputs):
    out, _ = run(inputs, N_STEPS)
    return out


# revision 16
# speedup vs baseline: 2.0324x; 2.0324x over previous
"""Tacotron2-style decoder on 8 Trainium2 NeuronCores.

Strategy (chosen over the data-parallel hint): tensor-parallel over the 4096
LSTM gate dims (512 gates/core) with weights resident in SBUF, attention
row-parallel (4 batch rows/core), two AllGathers per step (h_a; ctx).
Data-parallel would stream all 73MB of weights from HBM every step on every
core (they don't fit in SBUF) — memory-catastrophic for a 250-step recurrence.

kernel(**inputs) -> (mel_outputs, stop_tokens, attn_scores), matching
reference.reference().
"""
import numpy as np

import concourse.bass as bass
import concourse.mybir as mybir
import concourse.tile as tile
from concourse import bacc
from concourse import bass_utils

F32 = mybir.dt.float32
F32R = mybir.dt.float32r
AF = mybir.ActivationFunctionType

B, T_ENC, ENC_D = 32, 256, 512
T_DEC, MEL, R = 500, 80, 2
PRE = 256
ATTN_D, ATTN_RNN, DEC_RNN = 128, 1024, 1024
KSIZE = 31
PAD = (KSIZE - 1) // 2
NC = 8
RB = B // NC            # 4 rows per core
GS = 4 * ATTN_RNN // NC  # 512 gates per core
HS = ATTN_RNN // NC      # 128 h-dims per core
N_STEPS = T_DEC // R     # 250


def _r(x):  # operands are natively f32r now
    return x


def build_nc(n_steps):
    nc = bacc.Bacc("TRN2", target_bir_lowering=False, debug=False,
                   num_devices=NC)
    RG = [list(range(NC))]

    def inp(name, shape, dtype=F32):
        return nc.dram_tensor(name, list(shape), dtype, kind="ExternalInput")

    xT = inp("xT", (n_steps, 2, 128, B), F32R)
    WihaT = inp("WihaT", (128, 6, GS), F32R)
    WhhaT = inp("WhhaT", (128, 8, GS), F32R)
    WihdT = inp("WihdT", (128, 12, GS), F32R)
    WhhdT = inp("WhhdT", (128, 8, GS), F32R)
    ba_i = inp("ba", (1, GS), F32R)
    bd_i = inp("bd", (1, GS), F32R)
    WqT_i = inp("WqT", (128, 8, ATTN_D), F32R)
    WmsT_i = inp("WmsT", (128, 12, 162), F32R)
    bms_i = inp("bms", (1, 162), F32R)
    U31_i = inp("U31", (KSIZE, 128), F32R)
    wlcb_i = inp("wlcb", (128, 1))
    v_i = inp("vcol", (128, 2), F32R)
    enc_i = inp("enc_sb", (128, 8, ENC_D), F32R)
    pmT_i = inp("pmT", (128, RB, T_ENC))
    mask_i = inp("maskadd", (128, T_ENC))
    SEL_i = inp("SEL", (B, RB), F32R)
    ones_i = inp("ones", (1, B), F32R)
    ident_i = inp("ident", (128, 128))
    zz_i = inp("zz", (128, B), F32R)

    mel_o = nc.dram_tensor("mel_o", [n_steps, B, 161], F32,
                           kind="ExternalOutput")
    attn_o = nc.dram_tensor("attn_o", [n_steps, RB, T_ENC], F32,
                            kind="ExternalOutput")

    with tile.TileContext(nc) as tc:
        from contextlib import ExitStack
        stack = ExitStack()
        const_pool = stack.enter_context(tc.tile_pool(name="const", bufs=1))

        # ---------------- persistent SBUF ----------------
        def persist(name, shape, dtype=F32):
            return const_pool.tile(list(shape), dtype, name=name, tag=name)

        wiha = persist("wiha", (128, 6, GS), F32R)
        whha = persist("whha", (128, 8, GS), F32R)
        wihd = persist("wihd", (128, 12, GS), F32R)
        whhd = persist("whhd", (128, 8, GS), F32R)
        ba = persist("ba_s", (1, GS), F32R)
        bd = persist("bd_s", (1, GS), F32R)
        wqt = persist("wqt", (128, 8, ATTN_D), F32R)
        wmst = persist("wmst", (128, 12, 162), F32R)
        bms = persist("bms_s", (1, 162), F32R)
        u31 = persist("u31", (KSIZE, 128), F32R)
        wlcb = persist("wlcb_s", (128, 1))
        vcol = persist("vcol_s", (128, 2), F32R)
        enc = persist("enc_s", (128, 8, ENC_D), F32R)
        pmt = persist("pmt", (128, RB, T_ENC))
        mask = persist("mask_s", (128, T_ENC))
        sel = persist("sel_s", (B, RB), F32R)
        ones = persist("ones_s", (1, B), F32R)
        ident = persist("ident_s", (128, 128))
        zz = persist("zz_s", (128, B), F32R)

        haT = [persist(f"haT{i}", (128, 8, B), F32R) for i in range(2)]
        hdT = [persist(f"hdT{i}", (128, 8, B), F32R) for i in range(2)]
        ctxT = [persist(f"ctxT{i}", (128, RB, B), F32R) for i in range(2)]
        c_a = [persist(f"c_a{i}", (B, HS)) for i in range(2)]
        c_d = [persist(f"c_d{i}", (B, HS)) for i in range(2)]
        blk = [persist(f"blk{i}", (128, B), F32R) for i in range(2)]
        cum = persist("cum", (RB, T_ENC + 2 * PAD), F32R)

        # prologue loads
        for dst, src in [(wiha, WihaT), (whha, WhhaT), (wihd, WihdT),
                         (whhd, WhhdT), (ba, ba_i), (bd, bd_i),
                         (wqt, WqT_i), (wmst, WmsT_i), (bms, bms_i),
                         (u31, U31_i), (wlcb, wlcb_i), (vcol, v_i),
                         (enc, enc_i), (pmt, pmT_i), (mask, mask_i),
                         (sel, SEL_i), (ones, ones_i), (ident, ident_i),
                         (zz, zz_i)]:
            nc.sync.dma_start(dst[:], src[:])
        for t2 in haT + hdT + ctxT + c_a + c_d + blk + [cum]:
            nc.vector.memset(t2[:].bitcast(F32), 0.0)

        # ---------------- pools ----------------
        sb = stack.enter_context(tc.tile_pool(name="sb", bufs=2))
        sb3 = stack.enter_context(tc.tile_pool(name="sb3", bufs=3))
        ps_g = stack.enter_context(tc.tile_pool(name="ps_g", bufs=1,
                                                space="PSUM"))
        ps_att = stack.enter_context(tc.tile_pool(name="ps_att", bufs=1,
                                                  space="PSUM"))
        ps_sm = stack.enter_context(tc.tile_pool(name="ps_sm", bufs=1,
                                                 space="PSUM"))
        ps_eps = stack.enter_context(tc.tile_pool(name="ps_eps", bufs=2,
                                                  space="PSUM"))
        dram = stack.enter_context(tc.tile_pool(name="dram", bufs=3,
                                                space="DRAM"))

        ID32 = ident[0:32, 0:32]
        ID4 = ident[0:RB, 0:RB]

        def lstm_nl(gpsum, c_old, c_new, tag):
            """gates psum [B, 4*HS] -> h [B,HS] sbuf, c_new written."""
            i_s = sb.tile([B, HS], F32, name=f"i_{tag}", tag=f"i_{tag}")
            f_s = sb.tile([B, HS], F32, name=f"f_{tag}", tag=f"f_{tag}")
            g_t = sb.tile([B, HS], F32, name=f"g_{tag}", tag=f"g_{tag}")
            o_s = sb.tile([B, HS], F32, name=f"o_{tag}", tag=f"o_{tag}")
            tc_t = sb.tile([B, HS], F32, name=f"tc_{tag}", tag=f"tc_{tag}")
            h_sb = sb.tile([B, HS], F32, name=f"h_{tag}", tag=f"h_{tag}")
            nc.scalar.activation(i_s[:], gpsum[:, 0:HS], AF.Sigmoid)
            nc.scalar.activation(f_s[:], gpsum[:, HS:2 * HS], AF.Sigmoid)
            nc.scalar.activation(g_t[:], gpsum[:, 2 * HS:3 * HS], AF.Tanh)
            nc.scalar.activation(o_s[:], gpsum[:, 3 * HS:4 * HS], AF.Sigmoid)
            nc.vector.tensor_mul(f_s[:], f_s[:], c_old[:])
            nc.vector.tensor_mul(i_s[:], i_s[:], g_t[:])
            nc.vector.tensor_add(c_new[:], f_s[:], i_s[:])
            nc.scalar.activation(tc_t[:], c_new[:], AF.Tanh)
            nc.vector.tensor_mul(h_sb[:], o_s[:], tc_t[:])
            return h_sb

        agin1_next = dram.tile([2, 128, B], F32R, name="agin1", tag="agin1")
        nc.sync.dma_start(agin1_next[1], zz[:])   # h_d(-1) = 0

        for t in range(n_steps + 1):
            p, q = t % 2, (t + 1) % 2   # q = previous parity
            last = t == n_steps

            # ---- ploc precompute (needs cum(t), ready since t-1) ----
            if not last:
                cwin = sb.tile([KSIZE, RB, T_ENC], F32R, name="cwin",
                               tag="cwin")
                for j in range(RB):
                    base = cum[j:j + 1, 0:T_ENC]
                    src = bass.AP(base.tensor, base.offset,
                                  [list(base.ap[0]), [1, KSIZE], [1, T_ENC]])
                    nc.sync.dma_start(cwin[:, j, :], src)
                s_sb = sb.tile([128, RB, T_ENC], F32, name="s_sb", tag="s_sb")
                for h in range(2):
                    ploc = ps_att.tile([128, 2, T_ENC], F32, name="ploc",
                                       tag="ploc")
                    for jj in range(2):
                        j = 2 * h + jj
                        nc.tensor.matmul(ploc[:, jj, :], _r(u31[:]),
                                         _r(cwin[:, j, :]), start=True,
                                         stop=True)
                    for jj in range(2):
                        j = 2 * h + jj
                        nc.vector.tensor_add(s_sb[:, j, :], ploc[:, jj, :],
                                             pmt[:, j, :])

                # ---- gates_a ----
                g_a = ps_g.tile([B, GS], F32, name="g_a", tag="g_a")
                xts = sb3.tile([128, 2, B], F32R, name="xts", tag="xts")
                nc.sync.dma_start(xts[:], xT[t].transpose([1, 0, 2]))
                mm = []
                for k in range(2):
                    mm.append((xts[:, k, :], wiha[:, k, :]))
                for k in range(8):
                    mm.append((haT[q][:, k, :], whha[:, k, :]))
                mm.append((ones[:], ba[:]))
                for k in range(4):
                    mm.append((ctxT[q][:, k, :], wiha[:, 2 + k, :]))
                for i, (l, r_) in enumerate(mm):
                    nc.tensor.matmul(g_a[:], _r(l), _r(r_), start=(i == 0),
                                     stop=(i == len(mm) - 1))
                h_a = lstm_nl(g_a, c_a[q], c_a[p], "a")

                # transpose h_a -> [128, B], ship to AG1
                haTp = ps_sm.tile([128, B], F32, name="haTp", tag="sm")
                nc.tensor.transpose(haTp[:], h_a[:], ID32)
                haT_sb = sb.tile([128, B], F32R, name="haT_sb", tag="haT_sb")
                nc.vector.tensor_copy(haT_sb[:], haTp[:])
                agin1 = agin1_next
                nc.sync.dma_start(agin1[0], haT_sb[:])

                agout1 = dram.tile([NC, 2, 128, B], F32R, name="agout1",
                                   tag="agout1", addr_space="Shared")
                nc.gpsimd.collective_compute(
                    "AllGather", mybir.AluOpType.bypass, replica_groups=RG,
                    ins=[agin1[:].opt()], outs=[agout1[:].opt()])
                nc.sync.dma_start(haT[p][:],
                                  agout1[:, 0, :, :].transpose([1, 0, 2]))
                nc.sync.dma_start(hdT[p][:],
                                  agout1[:, 1, :, :].transpose([1, 0, 2]))
            else:
                agin1 = agin1_next
                nc.sync.dma_start(agin1[0], zz[:])
                agout1 = dram.tile([NC, 2, 128, B], F32R, name="agout1",
                                   tag="agout1", addr_space="Shared")
                nc.gpsimd.collective_compute(
                    "AllGather", mybir.AluOpType.bypass, replica_groups=RG,
                    ins=[agin1[:].opt()], outs=[agout1[:].opt()])
                nc.sync.dma_start(hdT[p][:],
                                  agout1[:, 1, :, :].transpose([1, 0, 2]))

            # ---- deferred mel/stop for step t-1 ----
            if t > 0:
                melp = ps_sm.tile([B, 162], F32, name="melp", tag="melp", bufs=1)
                mmm = [(hdT[p][:, k, :], wmst[:, k, :]) for k in range(8)]
                mmm += [(ctxT[q][:, k, :], wmst[:, 8 + k, :])
                        for k in range(4)]
                mmm.append((ones[:], bms[:]))
                for i, (l, r_) in enumerate(mmm):
                    nc.tensor.matmul(melp[:], _r(l), _r(r_), start=(i == 0),
                                     stop=(i == len(mmm) - 1))
                mel_sb = sb.tile([B, 162], F32, name="mel_sb", tag="mel_sb")
                nc.vector.tensor_copy(mel_sb[:, 0:160], melp[:, 0:160])
                nc.scalar.activation(mel_sb[:, 160:161], melp[:, 160:161],
                                     AF.Sigmoid)
                nc.sync.dma_start(mel_o[t - 1], mel_sb[:, 0:161])
            if last:
                break

            # ---- attention (own 4 rows) ----
            pqp = ps_sm.tile([B, ATTN_D], F32, name="pqp", tag="sm")
            for j in range(8):
                nc.tensor.matmul(pqp[:], _r(haT[p][:, j, :]),
                                 _r(wqt[:, j, :]), start=(j == 0),
                                 stop=(j == 7))
            pq_sb = sb.tile([B, ATTN_D], F32R, name="pq_sb", tag="pq_sb")
            nc.vector.tensor_copy(pq_sb[:], pqp[:])
            pqo = ps_sm.tile([RB, ATTN_D], F32, name="pqo", tag="sm")
            nc.tensor.matmul(pqo[:], _r(sel[:]), _r(pq_sb[:]), start=True,
                             stop=True)
            pqo_sb = sb.tile([RB, ATTN_D], F32, name="pqo_sb", tag="pqo_sb")
            nc.vector.tensor_copy(pqo_sb[:], pqo[:])
            pqT = ps_sm.tile([128, RB], F32, name="pqT", tag="sm")
            nc.tensor.transpose(pqT[:], pqo_sb[:], ID4)
            bias = sb.tile([128, RB], F32, name="bias", tag="bias")
            nc.vector.tensor_scalar_add(bias[:], pqT[:], wlcb[:])

            tanh_s = sb.tile([128, RB, T_ENC], F32R, name="tanh_s",
                             tag="tanh_s")
            for j in range(RB):
                nc.scalar.activation(tanh_s[:, j, :], s_sb[:, j, :], AF.Tanh,
                                     bias=bias[:, j:j + 1])
            e_sbB = sb.tile([128, T_ENC], F32, name="e_sbB", tag="e_sbB")
            for j in range(RB):
                e_ps = ps_eps.tile([2, T_ENC], F32, name="e_ps", tag="eps")
                nc.tensor.matmul(e_ps[:], _r(vcol[:]),
                                 _r(tanh_s[:, j, :]), start=True, stop=True)
                nc.vector.tensor_add(e_sbB[32 * j:32 * j + 1, :],
                                     e_ps[0:1, :],
                                     mask[32 * j:32 * j + 1, :])
            e_sb = sb.tile([RB, T_ENC], F32, name="e_sb", tag="e_sb")
            _eap = e_sbB[:]
            _gsrc = bass.AP(_eap.tensor, _eap.offset,
                            [[_eap.ap[0][0] * 32, RB], [1, T_ENC]])
            nc.sync.dma_start(e_sb[:], _gsrc)
            nmx = sb.tile([RB, 1], F32, name="nmx", tag="nmx")
            nc.vector.tensor_reduce(nmx[:], e_sb[:], mybir.AxisListType.X,
                                    mybir.AluOpType.max, negate=True)
            exp_sb = sb.tile([RB, T_ENC], F32, name="exp_sb", tag="exp_sb")
            esum = sb.tile([RB, 1], F32, name="esum", tag="esum")
            nc.scalar.activation(exp_sb[:], e_sb[:], AF.Exp, bias=nmx[:],
                                 accum_out=esum[:])
            rcp = sb.tile([RB, 1], F32, name="rcp", tag="rcp")
            nc.vector.reciprocal(rcp[:], esum[:])
            align = sb.tile([RB, T_ENC], F32, name="align", tag="align")
            nc.vector.tensor_scalar_mul(align[:], exp_sb[:], rcp[:])
            nc.sync.dma_start(attn_o[t], align[:])
            nc.vector.tensor_add(cum[:, PAD:PAD + T_ENC],
                                 cum[:, PAD:PAD + T_ENC], align[:])

            alT = ps_sm.tile([128, 2 * RB], F32, name="alT", tag="sm")
            for th in range(2):
                nc.tensor.transpose(alT[:, th * RB:(th + 1) * RB],
                                    align[:, th * 128:(th + 1) * 128], ID4)
            for j in range(8):
                b_i, th = j // 2, j % 2
                nc.vector.tensor_copy(blk[p][:, j * RB + b_i:j * RB + b_i + 1],
                                      alT[:, th * RB + b_i:th * RB + b_i + 1])
            ctxp = ps_att.tile([RB, ENC_D], F32, name="ctxp", tag="ctxp")
            for j in range(8):
                nc.tensor.matmul(ctxp[:], _r(blk[p][:, j * RB:(j + 1) * RB]),
                                 _r(enc[:, j, :]), start=(j == 0),
                                 stop=(j == 7))

            ctx_sb = sb.tile([RB, ENC_D], F32R, name="ctx_sb", tag="ctx_sb")
            nc.vector.tensor_copy(ctx_sb[:], ctxp[:])
            agin2 = dram.tile([RB, ENC_D], F32R, name="agin2", tag="agin2")
            nc.sync.dma_start(agin2[:], ctx_sb[:])
            agout2 = dram.tile([NC, RB, ENC_D], F32R, name="agout2",
                               tag="agout2", addr_space="Shared")
            nc.gpsimd.collective_compute(
                "AllGather", mybir.AluOpType.bypass, replica_groups=RG,
                ins=[agin2[:].opt()], outs=[agout2[:].opt()])
            # ctxT[p][pp, dj, r*RB+b] = agout2[r, b, dj*128+pp]
            for k in range(4):
                nc.sync.dma_start(
                    ctxT[p][:, k, :],
                    agout2[:, :, 128 * k:128 * (k + 1)]
                    .rearrange("r b pp -> pp (r b)"))

            # ---- gates_d ----
            g_d = ps_g.tile([B, GS], F32, name="g_d", tag="g_d")
            mm = [(haT[p][:, k, :], wihd[:, k, :]) for k in range(8)]
            mm += [(hdT[p][:, k, :], whhd[:, k, :]) for k in range(8)]
            mm.append((ones[:], bd[:]))
            mm += [(ctxT[p][:, k, :], wihd[:, 8 + k, :]) for k in range(4)]
            for i, (l, r_) in enumerate(mm):
                nc.tensor.matmul(g_d[:], _r(l), _r(r_), start=(i == 0),
                                 stop=(i == len(mm) - 1))
            h_d = lstm_nl(g_d, c_d[q], c_d[p], "d")
            hdTp = ps_sm.tile([128, B], F32, name="hdTp", tag="sm")
            nc.tensor.transpose(hdTp[:], h_d[:], ID32)
            hdT_sb = sb.tile([128, B], F32R, name="hdT_sb", tag="hdT_sb")
            nc.vector.tensor_copy(hdT_sb[:], hdTp[:])
            agin1_next = dram.tile([2, 128, B], F32R, name="agin1",
                                   tag="agin1")
            nc.sync.dma_start(agin1_next[1], hdT_sb[:])

        stack.close()

    nc.compile()
    return nc


# ---------------------------------------------------------------------------
# host-side prep
# ---------------------------------------------------------------------------
def prepare_in_maps(inputs, n_steps):
    f32 = np.float32

    def A(x):
        return np.ascontiguousarray(np.asarray(x, dtype=f32))

    enc_full = A(inputs['encoder_outputs'])
    x_in = A(inputs['inputs'])
    mlen = np.asarray(inputs['memory_lengths'])

    frames = np.zeros((N_STEPS, B, MEL), f32)
    frames[1:] = x_in[:, R - 1:T_DEC - 1:R, :].transpose(1, 0, 2)
    x = np.maximum(frames @ A(inputs['W1']).T + A(inputs['b1']), 0)
    x = np.maximum(x @ A(inputs['W2']).T + A(inputs['b2']), 0)
    xT = np.ascontiguousarray(
        x.transpose(0, 2, 1).reshape(N_STEPS, 2, 128, B)[:n_steps])

    def shard_rows(W, c):
        return np.concatenate(
            [W[q * ATTN_RNN + c * HS:q * ATTN_RNN + (c + 1) * HS]
             for q in range(4)], 0)

    Wih_a, Whh_a = A(inputs['Wih_a']), A(inputs['Whh_a'])
    Wih_d, Whh_d = A(inputs['Wih_d']), A(inputs['Whh_d'])
    ba_full = A(inputs['bih_a']) + A(inputs['bhh_a'])
    bd_full = A(inputs['bih_d']) + A(inputs['bhh_d'])
    Wm, WL = A(inputs['Wm']), A(inputs['WL'])
    conv_w, conv_b = A(inputs['conv_w']), A(inputs['conv_b'])
    Wq, v = A(inputs['Wq']), A(inputs['v'])
    Wms = np.concatenate([A(inputs['Wmel']), A(inputs['Wstop']),
                          np.zeros((1, 1536), f32)], 0)
    bms = np.concatenate([A(inputs['bmel']), A(inputs['bstop']),
                          np.zeros(1, f32)])[None]

    U31 = np.ascontiguousarray((WL @ conv_w[:, 0, :]).T)       # (31, 128)
    wlcb = np.ascontiguousarray((WL @ conv_b)[:, None])        # (128, 1)
    WqT = np.ascontiguousarray(
        Wq.T.reshape(8, 128, ATTN_D).transpose(1, 0, 2))       # (128,8,128)
    WmsT = np.ascontiguousarray(
        Wms.T.reshape(12, 128, 162).transpose(1, 0, 2))        # (128,12,162)

    shared = dict(xT=xT, ba=None, bd=None, WqT=WqT, WmsT=WmsT, bms=bms,
                  U31=U31, wlcb=wlcb, vcol=np.ascontiguousarray(np.concatenate([v[0][:, None], np.zeros((128, 1), f32)], 1)),
                  ones=np.ones((1, B), f32), ident=np.eye(128, dtype=f32),
                  zz=np.zeros((128, B), f32))

    in_maps = []
    for c in range(NC):
        rows = slice(c * RB, (c + 1) * RB)
        enc_own = enc_full[rows]
        enc_sb = np.ascontiguousarray(
            enc_own.reshape(RB * T_ENC, ENC_D).reshape(8, 128, ENC_D)
                   .transpose(1, 0, 2))
        pm = enc_own @ Wm.T                                    # (4,256,128)
        pmT = np.ascontiguousarray(pm.transpose(2, 0, 1))      # (128,4,256)
        mrows = np.where(
            np.arange(T_ENC)[None, :] >= np.asarray(mlen[rows])[:, None],
            np.float32(-1e9), np.float32(0.0)).astype(f32)
        maskadd = np.zeros((128, T_ENC), f32)
        maskadd[::32][:RB] = mrows
        SEL = np.zeros((B, RB), f32)
        for j in range(RB):
            SEL[c * RB + j, j] = 1.0

        def wt(W, nch):
            s = shard_rows(W, c)
            return np.ascontiguousarray(
                s.T.reshape(nch, 128, GS).transpose(1, 0, 2))

        m = dict(shared)
        m['WihaT'] = wt(Wih_a, 6)
        m['WhhaT'] = wt(Whh_a, 8)
        m['WihdT'] = wt(Wih_d, 12)
        m['WhhdT'] = wt(Whh_d, 8)
        m['ba'] = np.ascontiguousarray(np.concatenate(
            [ba_full[qq * ATTN_RNN + c * HS:qq * ATTN_RNN + (c + 1) * HS]
             for qq in range(4)])[None])
        m['bd'] = np.ascontiguousarray(np.concatenate(
            [bd_full[qq * ATTN_RNN + c * HS:qq * ATTN_RNN + (c + 1) * HS]
             for qq in range(4)])[None])
        m['enc_sb'] = enc_sb
        m['pmT'] = pmT
        m['maskadd'] = maskadd
        m['SEL'] = SEL
        in_maps.append(m)
    return in_maps


_CACHE = {}


def _get_nc(n_steps):
    if n_steps not in _CACHE:
        _CACHE[n_steps] = build_nc(n_steps)
    return _CACHE[n_steps]


def run(inputs, n_steps=N_STEPS, trace=False):
    nc = _get_nc(n_steps)
    in_maps = prepare_in_maps(inputs, n_steps)
    res = bass_utils.run_bass_kernel_spmd(
        nc, in_maps, core_ids=list(range(NC)), trace=trace)
    r = res.results
    mel_ms = r[0]['mel_o']                       # (n, 32, 161)
    mel = np.ascontiguousarray(
        mel_ms[:, :, :160].transpose(1, 0, 2).reshape(B, n_steps * R, MEL))
    stop = np.repeat(mel_ms[:, :, 160].T, R, axis=1)
    attn = np.concatenate([r[c]['attn_o'] for c in range(NC)], axis=1)
    attn = np.ascontiguousarray(attn.transpose(1, 0, 2))
    return (mel, stop, attn), res


def kernel(**inputs):
    out, _ = run(inputs, N_STEPS)
    return out


# revision 17
# speedup vs baseline: 4.1857x; 2.0595x over previous
"""Tacotron2-style decoder on 8 Trainium2 NeuronCores.

Strategy (chosen over the data-parallel hint): tensor-parallel over the 4096
LSTM gate dims (512 gates/core) with weights resident in SBUF, attention
row-parallel (4 batch rows/core), two AllGathers per step (h_a; ctx).
Data-parallel would stream all 73MB of weights from HBM every step on every
core (they don't fit in SBUF) — memory-catastrophic for a 250-step recurrence.

kernel(**inputs) -> (mel_outputs, stop_tokens, attn_scores), matching
reference.reference().
"""
import numpy as np

import concourse.bass as bass
import concourse.mybir as mybir
import concourse.tile as tile
from concourse import bacc
from concourse import bass_utils

F32 = mybir.dt.float32
F32R = mybir.dt.float32r
AF = mybir.ActivationFunctionType

B, T_ENC, ENC_D = 32, 256, 512
T_DEC, MEL, R = 500, 80, 2
PRE = 256
ATTN_D, ATTN_RNN, DEC_RNN = 128, 1024, 1024
KSIZE = 31
PAD = (KSIZE - 1) // 2
NC = 8
RB = B // NC            # 4 rows per core
GS = 4 * ATTN_RNN // NC  # 512 gates per core
HS = ATTN_RNN // NC      # 128 h-dims per core
N_STEPS = T_DEC // R     # 250


def _r(x):  # operands are natively f32r now
    return x


def build_nc(n_steps):
    nc = bacc.Bacc("TRN2", target_bir_lowering=False, debug=False,
                   num_devices=NC)
    RG = [list(range(NC))]

    def inp(name, shape, dtype=F32):
        return nc.dram_tensor(name, list(shape), dtype, kind="ExternalInput")

    xT = inp("xT", (n_steps, 2, 128, B), F32R)
    WihaT = inp("WihaT", (128, 6, GS), F32R)
    WhhaT = inp("WhhaT", (128, 8, GS), F32R)
    WihdT = inp("WihdT", (128, 12, GS), F32R)
    WhhdT = inp("WhhdT", (128, 8, GS), F32R)
    ba_i = inp("ba", (1, GS), F32R)
    bd_i = inp("bd", (1, GS), F32R)
    WqT_i = inp("WqT", (128, 8, ATTN_D), F32R)
    WmsT_i = inp("WmsT", (128, 12, 162), F32R)
    bms_i = inp("bms", (1, 162), F32R)
    U31_i = inp("U31", (KSIZE, 128), F32R)
    wlcb_i = inp("wlcb", (128, 1))
    v_i = inp("vcol", (128, 2), F32R)
    enc_i = inp("enc_sb", (128, 8, ENC_D), F32R)
    pmT_i = inp("pmT", (128, RB, T_ENC))
    mask_i = inp("maskadd", (128, T_ENC))
    SEL_i = inp("SEL", (B, RB), F32R)
    ones_i = inp("ones", (1, B), F32R)
    ident_i = inp("ident", (128, 128))
    zz_i = inp("zz", (128, B), F32R)

    mel_o = nc.dram_tensor("mel_o", [n_steps, B, 161], F32,
                           kind="ExternalOutput")
    attn_o = nc.dram_tensor("attn_o", [n_steps, RB, T_ENC], F32,
                            kind="ExternalOutput")

    with tile.TileContext(nc) as tc:
        from contextlib import ExitStack
        stack = ExitStack()
        const_pool = stack.enter_context(tc.tile_pool(name="const", bufs=1))

        # ---------------- persistent SBUF ----------------
        def persist(name, shape, dtype=F32):
            return const_pool.tile(list(shape), dtype, name=name, tag=name)

        wiha = persist("wiha", (128, 6, GS), F32R)
        whha = persist("whha", (128, 8, GS), F32R)
        wihd = persist("wihd", (128, 12, GS), F32R)
        whhd = persist("whhd", (128, 8, GS), F32R)
        ba = persist("ba_s", (1, GS), F32R)
        bd = persist("bd_s", (1, GS), F32R)
        wqt = persist("wqt", (128, 8, ATTN_D), F32R)
        wmst = persist("wmst", (128, 12, 162), F32R)
        bms = persist("bms_s", (1, 162), F32R)
        u31 = persist("u31", (KSIZE, 128), F32R)
        wlcb = persist("wlcb_s", (128, 1))
        vcol = persist("vcol_s", (128, 2), F32R)
        enc = persist("enc_s", (128, 8, ENC_D), F32R)
        pmt = persist("pmt", (128, RB, T_ENC))
        mask = persist("mask_s", (128, T_ENC))
        sel = persist("sel_s", (B, RB), F32R)
        ones = persist("ones_s", (1, B), F32R)
        ident = persist("ident_s", (128, 128))
        zz = persist("zz_s", (128, B), F32R)

        haT = [persist(f"haT{i}", (128, 8, B), F32R) for i in range(2)]
        hdT = [persist(f"hdT{i}", (128, 8, B), F32R) for i in range(2)]
        ctxT = [persist(f"ctxT{i}", (128, RB, B), F32R) for i in range(2)]
        c_a = [persist(f"c_a{i}", (B, HS)) for i in range(2)]
        c_d = [persist(f"c_d{i}", (B, HS)) for i in range(2)]
        blk = [persist(f"blk{i}", (128, B), F32R) for i in range(2)]
        cum = persist("cum", (RB, T_ENC + 2 * PAD), F32R)

        # prologue loads
        for dst, src in [(wiha, WihaT), (whha, WhhaT), (wihd, WihdT),
                         (whhd, WhhdT), (ba, ba_i), (bd, bd_i),
                         (wqt, WqT_i), (wmst, WmsT_i), (bms, bms_i),
                         (u31, U31_i), (wlcb, wlcb_i), (vcol, v_i),
                         (enc, enc_i), (pmt, pmT_i), (mask, mask_i),
                         (sel, SEL_i), (ones, ones_i), (ident, ident_i),
                         (zz, zz_i)]:
            nc.sync.dma_start(dst[:], src[:])
        for t2 in haT + hdT + ctxT + c_a + c_d + blk + [cum]:
            nc.vector.memset(t2[:].bitcast(F32), 0.0)

        # ---------------- pools ----------------
        sb = stack.enter_context(tc.tile_pool(name="sb", bufs=2))
        sb3 = stack.enter_context(tc.tile_pool(name="sb3", bufs=3))
        ps_g = stack.enter_context(tc.tile_pool(name="ps_g", bufs=1,
                                                space="PSUM"))
        ps_att = stack.enter_context(tc.tile_pool(name="ps_att", bufs=1,
                                                  space="PSUM"))
        ps_sm = stack.enter_context(tc.tile_pool(name="ps_sm", bufs=1,
                                                 space="PSUM"))
        ps_eps = stack.enter_context(tc.tile_pool(name="ps_eps", bufs=2,
                                                  space="PSUM"))
        dram = stack.enter_context(tc.tile_pool(name="dram", bufs=3,
                                                space="DRAM"))

        ID32 = ident[0:32, 0:32]
        ID4 = ident[0:RB, 0:RB]

        def lstm_nl(gpsum, c_old, c_new, tag):
            """gates psum [B, 4*HS] -> h [B,HS] sbuf, c_new written."""
            i_s = sb.tile([B, HS], F32, name=f"i_{tag}", tag=f"i_{tag}")
            f_s = sb.tile([B, HS], F32, name=f"f_{tag}", tag=f"f_{tag}")
            g_t = sb.tile([B, HS], F32, name=f"g_{tag}", tag=f"g_{tag}")
            o_s = sb.tile([B, HS], F32, name=f"o_{tag}", tag=f"o_{tag}")
            tc_t = sb.tile([B, HS], F32, name=f"tc_{tag}", tag=f"tc_{tag}")
            h_sb = sb.tile([B, HS], F32, name=f"h_{tag}", tag=f"h_{tag}")
            nc.scalar.activation(i_s[:], gpsum[:, 0:HS], AF.Sigmoid)
            nc.scalar.activation(f_s[:], gpsum[:, HS:2 * HS], AF.Sigmoid)
            nc.scalar.activation(g_t[:], gpsum[:, 2 * HS:3 * HS], AF.Tanh)
            nc.scalar.activation(o_s[:], gpsum[:, 3 * HS:4 * HS], AF.Sigmoid)
            nc.vector.tensor_mul(f_s[:], f_s[:], c_old[:])
            nc.vector.tensor_mul(i_s[:], i_s[:], g_t[:])
            nc.vector.tensor_add(c_new[:], f_s[:], i_s[:])
            nc.scalar.activation(tc_t[:], c_new[:], AF.Tanh)
            nc.vector.tensor_mul(h_sb[:], o_s[:], tc_t[:])
            return h_sb

        melctx = {}

        def do_mel():
            t, p, q = melctx['t'], melctx['p'], melctx['q']
            if t <= 0:
                return
            melp = ps_sm.tile([B, 162], F32, name="melp", tag="melp",
                              bufs=1)
            mmm = [(hdT[p][:, k, :], wmst[:, k, :]) for k in range(8)]
            mmm += [(ctxT[q][:, k, :], wmst[:, 8 + k, :]) for k in range(4)]
            mmm.append((ones[:], bms[:]))
            for i, (l, r_) in enumerate(mmm):
                nc.tensor.matmul(melp[:], _r(l), _r(r_), start=(i == 0),
                                 stop=(i == len(mmm) - 1))
            mel_sb = sb.tile([B, 162], F32, name="mel_sb", tag="mel_sb")
            nc.vector.tensor_copy(mel_sb[:, 0:160], melp[:, 0:160])
            nc.scalar.activation(mel_sb[:, 160:161], melp[:, 160:161],
                                 AF.Sigmoid)
            nc.sync.dma_start(mel_o[t - 1], mel_sb[:, 0:161])

        agin1_next = dram.tile([2, 128, B], F32R, name="agin1", tag="agin1")
        nc.sync.dma_start(agin1_next[1], zz[:])   # h_d(-1) = 0

        for t in range(n_steps + 1):
            p, q = t % 2, (t + 1) % 2   # q = previous parity
            last = t == n_steps
            melctx.update(t=t, p=p, q=q)

            # ---- ploc precompute (needs cum(t), ready since t-1) ----
            if not last:
                cwin = sb.tile([KSIZE, RB, T_ENC], F32R, name="cwin",
                               tag="cwin")
                for j in range(RB):
                    base = cum[j:j + 1, 0:T_ENC]
                    src = bass.AP(base.tensor, base.offset,
                                  [list(base.ap[0]), [1, KSIZE], [1, T_ENC]])
                    nc.sync.dma_start(cwin[:, j, :], src)
                s_sb = sb.tile([128, RB, T_ENC], F32, name="s_sb", tag="s_sb")
                for h in range(2):
                    ploc = ps_att.tile([128, 2, T_ENC], F32, name="ploc",
                                       tag="ploc")
                    for jj in range(2):
                        j = 2 * h + jj
                        nc.tensor.matmul(ploc[:, jj, :], _r(u31[:]),
                                         _r(cwin[:, j, :]), start=True,
                                         stop=True)
                    for jj in range(2):
                        j = 2 * h + jj
                        nc.vector.tensor_add(s_sb[:, j, :], ploc[:, jj, :],
                                             pmt[:, j, :])

                # ---- gates_a ----
                g_a = ps_g.tile([B, GS], F32, name="g_a", tag="g_a")
                xts = sb3.tile([128, 2, B], F32R, name="xts", tag="xts")
                nc.sync.dma_start(xts[:], xT[t].transpose([1, 0, 2]))
                mm = []
                for k in range(2):
                    mm.append((xts[:, k, :], wiha[:, k, :]))
                for k in range(8):
                    mm.append((haT[q][:, k, :], whha[:, k, :]))
                mm.append((ones[:], ba[:]))
                for k in range(4):
                    mm.append((ctxT[q][:, k, :], wiha[:, 2 + k, :]))
                for i, (l, r_) in enumerate(mm):
                    nc.tensor.matmul(g_a[:], _r(l), _r(r_), start=(i == 0),
                                     stop=(i == len(mm) - 1))
                h_a = lstm_nl(g_a, c_a[q], c_a[p], "a")

                # transpose h_a -> [128, B], ship to AG1
                haTp = ps_sm.tile([128, B], F32, name="haTp", tag="sm")
                nc.tensor.transpose(haTp[:], h_a[:], ID32)
                haT_sb = sb.tile([128, B], F32R, name="haT_sb", tag="haT_sb")
                nc.vector.tensor_copy(haT_sb[:], haTp[:])
                agin1 = agin1_next
                nc.sync.dma_start(agin1[0], haT_sb[:])

                agout1 = dram.tile([NC, 2, 128, B], F32R, name="agout1",
                                   tag="agout1", addr_space="Shared")
                nc.gpsimd.collective_compute(
                    "AllGather", mybir.AluOpType.bypass, replica_groups=RG,
                    ins=[agin1[:].opt()], outs=[agout1[:].opt()])
                nc.sync.dma_start(haT[p][:],
                                  agout1[:, 0, :, :].transpose([1, 0, 2]))
                nc.sync.dma_start(hdT[p][:],
                                  agout1[:, 1, :, :].transpose([1, 0, 2]))
            else:
                agin1 = agin1_next
                nc.sync.dma_start(agin1[0], zz[:])
                agout1 = dram.tile([NC, 2, 128, B], F32R, name="agout1",
                                   tag="agout1", addr_space="Shared")
                nc.gpsimd.collective_compute(
                    "AllGather", mybir.AluOpType.bypass, replica_groups=RG,
                    ins=[agin1[:].opt()], outs=[agout1[:].opt()])
                nc.sync.dma_start(hdT[p][:],
                                  agout1[:, 1, :, :].transpose([1, 0, 2]))

            # ---- deferred mel/stop for step t-1 (epilogue only here) ----
            if last:
                do_mel()
                break

            # ---- attention (own 4 rows) ----
            pqp = ps_sm.tile([B, ATTN_D], F32, name="pqp", tag="sm")
            for j in range(8):
                nc.tensor.matmul(pqp[:], _r(haT[p][:, j, :]),
                                 _r(wqt[:, j, :]), start=(j == 0),
                                 stop=(j == 7))
            pq_sb = sb.tile([B, ATTN_D], F32R, name="pq_sb", tag="pq_sb")
            nc.vector.tensor_copy(pq_sb[:], pqp[:])
            pqo = ps_sm.tile([RB, ATTN_D], F32, name="pqo", tag="sm")
            nc.tensor.matmul(pqo[:], _r(sel[:]), _r(pq_sb[:]), start=True,
                             stop=True)
            pqo_sb = sb.tile([RB, ATTN_D], F32, name="pqo_sb", tag="pqo_sb")
            nc.vector.tensor_copy(pqo_sb[:], pqo[:])
            pqT = ps_sm.tile([128, RB], F32, name="pqT", tag="sm")
            nc.tensor.transpose(pqT[:], pqo_sb[:], ID4)
            bias = sb.tile([128, RB], F32, name="bias", tag="bias")
            nc.vector.tensor_scalar_add(bias[:], pqT[:], wlcb[:])

            tanh_s = sb.tile([128, RB, T_ENC], F32R, name="tanh_s",
                             tag="tanh_s")
            for j in range(RB):
                nc.scalar.activation(tanh_s[:, j, :], s_sb[:, j, :], AF.Tanh,
                                     bias=bias[:, j:j + 1])
            e_sbB = sb.tile([128, T_ENC], F32, name="e_sbB", tag="e_sbB")
            for j in range(RB):
                e_ps = ps_eps.tile([2, T_ENC], F32, name="e_ps", tag="eps")
                nc.tensor.matmul(e_ps[:], _r(vcol[:]),
                                 _r(tanh_s[:, j, :]), start=True, stop=True)
                nc.vector.tensor_add(e_sbB[32 * j:32 * j + 1, :],
                                     e_ps[0:1, :],
                                     mask[32 * j:32 * j + 1, :])
            e_sb = sb.tile([RB, T_ENC], F32, name="e_sb", tag="e_sb")
            _eap = e_sbB[:]
            _gsrc = bass.AP(_eap.tensor, _eap.offset,
                            [[_eap.ap[0][0] * 32, RB], [1, T_ENC]])
            nc.sync.dma_start(e_sb[:], _gsrc)
            nmx = sb.tile([RB, 1], F32, name="nmx", tag="nmx")
            nc.vector.tensor_reduce(nmx[:], e_sb[:], mybir.AxisListType.X,
                                    mybir.AluOpType.max, negate=True)
            exp_sb = sb.tile([RB, T_ENC], F32, name="exp_sb", tag="exp_sb")
            esum = sb.tile([RB, 1], F32, name="esum", tag="esum")
            nc.scalar.activation(exp_sb[:], e_sb[:], AF.Exp, bias=nmx[:],
                                 accum_out=esum[:])
            rcp = sb.tile([RB, 1], F32, name="rcp", tag="rcp")
            nc.vector.reciprocal(rcp[:], esum[:])
            align = sb.tile([RB, T_ENC], F32, name="align", tag="align")
            nc.vector.tensor_scalar_mul(align[:], exp_sb[:], rcp[:])
            alT = ps_sm.tile([128, 2 * RB], F32, name="alT", tag="sm")
            for th in range(2):
                nc.tensor.transpose(alT[:, th * RB:(th + 1) * RB],
                                    align[:, th * 128:(th + 1) * 128], ID4)
            # blk col j*4+j//2 <- alT col th*4+b (j=2b+th): two strided copies
            bap = blk[p][:]
            for th in range(2):
                dst = bass.AP(bap.tensor, bap.offset + th * RB,
                              [list(bap.ap[0]), [9, RB]])
                nc.vector.tensor_copy(dst, alT[:, th * RB:(th + 1) * RB])
            nc.sync.dma_start(attn_o[t], align[:])
            nc.vector.tensor_add(cum[:, PAD:PAD + T_ENC],
                                 cum[:, PAD:PAD + T_ENC], align[:])
            ctxp = ps_att.tile([RB, ENC_D], F32, name="ctxp", tag="ctxp")
            for j in range(8):
                nc.tensor.matmul(ctxp[:], _r(blk[p][:, j * RB:(j + 1) * RB]),
                                 _r(enc[:, j, :]), start=(j == 0),
                                 stop=(j == 7))

            ctx_sb = sb.tile([RB, ENC_D], F32R, name="ctx_sb", tag="ctx_sb")
            nc.vector.tensor_copy(ctx_sb[:], ctxp[:])
            agin2 = dram.tile([RB, ENC_D], F32R, name="agin2", tag="agin2")
            nc.sync.dma_start(agin2[:], ctx_sb[:])
            do_mel()
            agout2 = dram.tile([NC, RB, ENC_D], F32R, name="agout2",
                               tag="agout2", addr_space="Shared")
            nc.gpsimd.collective_compute(
                "AllGather", mybir.AluOpType.bypass, replica_groups=RG,
                ins=[agin2[:].opt()], outs=[agout2[:].opt()])
            # ctxT[p][pp, dj, r*RB+b] = agout2[r, b, dj*128+pp]
            for k in range(4):
                nc.sync.dma_start(
                    ctxT[p][:, k, :],
                    agout2[:, :, 128 * k:128 * (k + 1)]
                    .rearrange("r b pp -> pp (r b)"))

            # ---- gates_d ----
            g_d = ps_g.tile([B, GS], F32, name="g_d", tag="g_d")
            mm = [(haT[p][:, k, :], wihd[:, k, :]) for k in range(8)]
            mm += [(hdT[p][:, k, :], whhd[:, k, :]) for k in range(8)]
            mm.append((ones[:], bd[:]))
            mm += [(ctxT[p][:, k, :], wihd[:, 8 + k, :]) for k in range(4)]
            for i, (l, r_) in enumerate(mm):
                nc.tensor.matmul(g_d[:], _r(l), _r(r_), start=(i == 0),
                                 stop=(i == len(mm) - 1))
            h_d = lstm_nl(g_d, c_d[q], c_d[p], "d")
            hdTp = ps_sm.tile([128, B], F32, name="hdTp", tag="sm")
            nc.tensor.transpose(hdTp[:], h_d[:], ID32)
            hdT_sb = sb.tile([128, B], F32R, name="hdT_sb", tag="hdT_sb")
            nc.vector.tensor_copy(hdT_sb[:], hdTp[:])
            agin1_next = dram.tile([2, 128, B], F32R, name="agin1",
                                   tag="agin1")
            nc.sync.dma_start(agin1_next[1], hdT_sb[:])

        stack.close()

    nc.compile()
    return nc


# ---------------------------------------------------------------------------
# host-side prep
# ---------------------------------------------------------------------------
def prepare_in_maps(inputs, n_steps):
    f32 = np.float32

    def A(x):
        return np.ascontiguousarray(np.asarray(x, dtype=f32))

    enc_full = A(inputs['encoder_outputs'])
    x_in = A(inputs['inputs'])
    mlen = np.asarray(inputs['memory_lengths'])

    frames = np.zeros((N_STEPS, B, MEL), f32)
    frames[1:] = x_in[:, R - 1:T_DEC - 1:R, :].transpose(1, 0, 2)
    x = np.maximum(frames @ A(inputs['W1']).T + A(inputs['b1']), 0)
    x = np.maximum(x @ A(inputs['W2']).T + A(inputs['b2']), 0)
    xT = np.ascontiguousarray(
        x.transpose(0, 2, 1).reshape(N_STEPS, 2, 128, B)[:n_steps])

    def shard_rows(W, c):
        return np.concatenate(
            [W[q * ATTN_RNN + c * HS:q * ATTN_RNN + (c + 1) * HS]
             for q in range(4)], 0)

    Wih_a, Whh_a = A(inputs['Wih_a']), A(inputs['Whh_a'])
    Wih_d, Whh_d = A(inputs['Wih_d']), A(inputs['Whh_d'])
    ba_full = A(inputs['bih_a']) + A(inputs['bhh_a'])
    bd_full = A(inputs['bih_d']) + A(inputs['bhh_d'])
    Wm, WL = A(inputs['Wm']), A(inputs['WL'])
    conv_w, conv_b = A(inputs['conv_w']), A(inputs['conv_b'])
    Wq, v = A(inputs['Wq']), A(inputs['v'])
    Wms = np.concatenate([A(inputs['Wmel']), A(inputs['Wstop']),
                          np.zeros((1, 1536), f32)], 0)
    bms = np.concatenate([A(inputs['bmel']), A(inputs['bstop']),
                          np.zeros(1, f32)])[None]

    U31 = np.ascontiguousarray((WL @ conv_w[:, 0, :]).T)       # (31, 128)
    wlcb = np.ascontiguousarray((WL @ conv_b)[:, None])        # (128, 1)
    WqT = np.ascontiguousarray(
        Wq.T.reshape(8, 128, ATTN_D).transpose(1, 0, 2))       # (128,8,128)
    WmsT = np.ascontiguousarray(
        Wms.T.reshape(12, 128, 162).transpose(1, 0, 2))        # (128,12,162)

    shared = dict(xT=xT, ba=None, bd=None, WqT=WqT, WmsT=WmsT, bms=bms,
                  U31=U31, wlcb=wlcb, vcol=np.ascontiguousarray(np.concatenate([v[0][:, None], np.zeros((128, 1), f32)], 1)),
                  ones=np.ones((1, B), f32), ident=np.eye(128, dtype=f32),
                  zz=np.zeros((128, B), f32))

    in_maps = []
    for c in range(NC):
        rows = slice(c * RB, (c + 1) * RB)
        enc_own = enc_full[rows]
        enc_sb = np.ascontiguousarray(
            enc_own.reshape(RB * T_ENC, ENC_D).reshape(8, 128, ENC_D)
                   .transpose(1, 0, 2))
        pm = enc_own @ Wm.T                                    # (4,256,128)
        pmT = np.ascontiguousarray(pm.transpose(2, 0, 1))      # (128,4,256)
        mrows = np.where(
            np.arange(T_ENC)[None, :] >= np.asarray(mlen[rows])[:, None],
            np.float32(-1e9), np.float32(0.0)).astype(f32)
        maskadd = np.zeros((128, T_ENC), f32)
        maskadd[::32][:RB] = mrows
        SEL = np.zeros((B, RB), f32)
        for j in range(RB):
            SEL[c * RB + j, j] = 1.0

        def wt(W, nch):
            s = shard_rows(W, c)
            return np.ascontiguousarray(
                s.T.reshape(nch, 128, GS).transpose(1, 0, 2))

        m = dict(shared)
        m['WihaT'] = wt(Wih_a, 6)
        m['WhhaT'] = wt(Whh_a, 8)
        m['WihdT'] = wt(Wih_d, 12)
        m['WhhdT'] = wt(Whh_d, 8)
        m['ba'] = np.ascontiguousarray(np.concatenate(
            [ba_full[qq * ATTN_RNN + c * HS:qq * ATTN_RNN + (c + 1) * HS]
             for qq in range(4)])[None])
        m['bd'] = np.ascontiguousarray(np.concatenate(
            [bd_full[qq * ATTN_RNN + c * HS:qq * ATTN_RNN + (c + 1) * HS]
             for qq in range(4)])[None])
        m['enc_sb'] = enc_sb
        m['pmT'] = pmT
        m['maskadd'] = maskadd
        m['SEL'] = SEL
        in_maps.append(m)
    return in_maps


_CACHE = {}


def _get_nc(n_steps):
    if n_steps not in _CACHE:
        _CACHE[n_steps] = build_nc(n_steps)
    return _CACHE[n_steps]


def run(inputs, n_steps=N_STEPS, trace=False):
    nc = _get_nc(n_steps)
    in_maps = prepare_in_maps(inputs, n_steps)
    res = bass_utils.run_bass_kernel_spmd(
        nc, in_maps, core_ids=list(range(NC)), trace=trace)
    r = res.results
    mel_ms = r[0]['mel_o']                       # (n, 32, 161)
    mel = np.ascontiguousarray(
        mel_ms[:, :, :160].transpose(1, 0, 2).reshape(B, n_steps * R, MEL))
    stop = np.repeat(mel_ms[:, :, 160].T, R, axis=1)
    attn = np.concatenate([r[c]['attn_o'] for c in range(NC)], axis=1)
    attn = np.ascontiguousarray(attn.transpose(1, 0, 2))
    return (mel, stop, attn), res


def kernel(**inputs):
    out, _ = run(inputs, N_STEPS)
    return out
